# revision 62
# baseline (speedup 1.0000x reference)
"""Multi-head attention (B=2, S=2048, D=1024, H=16, no mask) on 8 TRN2 cores.

Sharding: tensor-parallel over heads — 2 heads per core. Each core computes
its heads' QKV projections, attention, and a partial out-projection
(row-sharded Wo); the 8 partials are summed on device and the host adds bo.

v3 (~182us TimelineSim vs the ~232us v2 baseline, rel err ~8e-3):
  - QK runs in fp8e4 DoubleRow perf mode (0.5 PE cycles/column): k is a
    single-fp8 stationary duplicated across the two DoubleRow planes via
    a stride-0 AP; q is stored hi/lo (fp8 value + fp8 residual) in the
    two moving planes, so the product is k8^T(q_hi+q_lo) — only k
    carries fp8 noise (~1.2% through softmax). q/k are scaled 4x into
    the e4m3 sweet range; 1/(sqrt(HD)*16) folds into the ACT exp scale.
  - the sk loop is software-pipelined: exp(sk) issues first, QK(sk+1)
    right after (one step of lookahead), then PV(sk-1), then injected
    filler; the next q-block's first QK+exp are pre-issued at this
    block's tail so ACT (the pacer: 128 x ~1.04us exps) never starves.
  - filler work (projections, va builds, outproj, normalize) is a
    deadline-tagged unit queue: pop_units force-pops anything due at the
    current step (a missed deadline = a consumer reading an unwritten
    tile region = silent garbage), and paces the rest across the batch.
  - v is projected TRANSPOSED (x sk-tile as the stationary operand), so
    the [token, vfeature] PSUM result fills the ones-augmented va tiles
    with a single strided copy — no transpose pass.
  - x loads: k-tile-pair DMAs for chunk 0 (pipelines into the exposed
    projections), merged whole-row DMAs for chunks 1-3 and batch 1; the
    scalar (ACT) ring carries only wv+wq (each DMA trigger costs ~0.6us
    of the shared HWDGE generator and would delay the first exp).
  - softmax normalize: ctx PSUM is evacuated once per head (frees the
    bank), the raw denominator row is partition-broadcast with one K=1
    PE matmul, reciprocated as a [64,512] DVE op (accurate on positive
    denominators), and multiplied in. Out-projection partials ship bf16.
"""
import numpy as np

B = 2
S = 2048
D = 1024
H = 16
HD = 64
NCORES = 8
HPC = H // NCORES       # heads per core
FPC = HPC * HD          # 128 features per core
QKSCALE = 4.0           # q/k fp8 range scaling (folded out in exp scale)
EXPSCALE = 1.0 / (8.0 * QKSCALE * QKSCALE)


def build_mha_kernel(tc, outT, xT, wqT, wkT, wvT, woT, s=S, d=D):
    """Emit the per-core MHA program.

    outT: [B, d, s] f32 (partial output, transposed, per-batch)
    xT:   [B, d, s] bf16 (host pre-transposed)
    wqT/wkT/wvT: [128, d//128, FPC] bf16, host pre-arranged so the
        weight DMA is contiguous (wqT/wkT pre-scaled by QKSCALE)
    woT:  [FPC, d] f32r
    """
    import concourse.mybir as mybir
    from concourse.masks import make_identity
    from contextlib import ExitStack

    nc = tc.nc
    f32 = mybir.dt.float32
    f32r = mybir.dt.float32r
    bf16 = mybir.dt.bfloat16
    fp8 = mybir.dt.float8e4
    Exp = mybir.ActivationFunctionType.Exp
    DR = mybir.MatmulPerfMode.DoubleRow
    MUL = mybir.AluOpType.mult
    SUB = mybir.AluOpType.subtract

    KT = d // 128           # contraction tiles for projections
    SK = s // 128           # key tiles
    NQB = s // 512          # 512-col query blocks
    NCH = s // 512          # 512-token chunks

    with ExitStack() as es:
        consts = es.enter_context(tc.tile_pool(name="consts", bufs=1))
        wpool = es.enter_context(tc.tile_pool(name="w", bufs=1))
        xpool = es.enter_context(tc.tile_pool(name="xt", bufs=1))
        qkv = es.enter_context(tc.tile_pool(name="qkv", bufs=1))
        vapool = es.enter_context(tc.tile_pool(name="va", bufs=1))
        epool = es.enter_context(tc.tile_pool(name="exp", bufs=3))
        cpool = es.enter_context(tc.tile_pool(name="ctxT", bufs=1))
        spool = es.enter_context(tc.tile_pool(name="small", bufs=2))
        opool = es.enter_context(tc.tile_pool(name="o", bufs=6))
        ps_ctx = es.enter_context(tc.tile_pool(name="psctx", bufs=1, space="PSUM"))
        ps_sc = es.enter_context(tc.tile_pool(name="pssc", bufs=2, space="PSUM"))
        ps_wk = es.enter_context(tc.tile_pool(name="pswk", bufs=2, space="PSUM"))

        identity = consts.tile([128, 128], f32, tag="ident")
        make_identity(nc, identity[:])
        ident_r = consts.tile([128, 128], f32r, tag="ident_r")
        nc.vector.tensor_copy(ident_r[:], identity[:])
        ones_f = consts.tile([128, 1], f32, tag="ones_f")
        nc.gpsimd.memset(ones_f[:], 1.0)
        # bc stationary: row 64 of a [65, 64] tile is all-ones so the K=1
        # denominator-broadcast matmul's lhsT sits at partition 64 (same
        # base partition as the denominator row in cu).
        ones65_f = consts.tile([65, HD], f32, tag="ones65_f")
        nc.gpsimd.memset(ones65_f[:], 1.0)
        ones65 = consts.tile([65, HD], f32r, tag="ones65")
        nc.vector.tensor_copy(ones65[:], ones65_f[:])
        # shf64: sh[64+j,q] = cn[j,q]  (normalized h1 ctx -> lanes 64-127)
        shf_f = consts.tile([HD, 2 * HD], f32, tag="shf_f")
        nc.gpsimd.memset(shf_f[:], 0.0)
        nc.vector.tensor_copy(shf_f[0:HD, HD:2 * HD], identity[0:HD, 0:HD])
        shf64 = consts.tile([HD, 2 * HD], f32r, tag="shf64")
        nc.vector.tensor_copy(shf64[:], shf_f[:])

        # --- weights (resident for the whole kernel). wk/wv go first on the
        # two HWDGE rings; wq after batch 0's chunk-0 x tiles; wo later.
        wk_sb = wpool.tile([128, KT, FPC], bf16, tag="wk")
        nc.sync.dma_start(wk_sb[:], wkT)
        wv_sb = wpool.tile([128, KT, FPC], bf16, tag="wv")
        nc.scalar.dma_start(wv_sb[:], wvT)
        wq_sb = wpool.tile([128, KT, FPC], bf16, tag="wq")
        # wq trigger rides early on the scalar ring: HWDGE serves rings
        # round-robin, so a late wq sits behind the x tails and stalls the
        # exposed q projection (~2us of PE idle at startup)
        nc.scalar.dma_start(wq_sb[:], wqT)
        wo_sb = wpool.tile([128, d], f32r, tag="wo")

        # --- PE p-state warm-up: the PE runs at mid speed until it has a
        # ~3us continuous execution streak. These dummy matmuls burn the
        # otherwise-idle DMA-wait window so the exposed chunk-0 projections
        # (and everything after) run at full speed sooner.
        warm = ps_wk.tile([128, 128], f32, tag="wk")
        for _ in range(14):
            nc.tensor.matmul(warm[:], ident_r[:], ident_r[:],
                             start=True, stop=True)

        # --- persistent ones-augmented v tiles, both heads per tile:
        # columns [h0 v(64) | ones | h1 v(64) | ones].
        va_tiles = {}
        for g in range(2):
            for sk in range(SK):
                va = vapool.tile([128, 2 * (HD + 1)], f32r,
                                 tag=f"va{sk}_{g}", name=f"va{sk}_{g}")
                for h in range(HPC):
                    # on Pool: 64 tiny copies would clog the DVE queue head
                    # and delay the chunk-0 projection evacuations
                    nc.gpsimd.tensor_copy(
                        va[:, h * (HD + 1) + HD:h * (HD + 1) + HD + 1],
                        ones_f[:])
                va_tiles[(g, sk)] = va

        pending = []
        op_carry = [[]]
        next_late = [[]]
        next_x_emitters = []

        def pop_units(steps_left, cur_step=None):
            # pop at an adaptive base rate, but ALWAYS pop units whose
            # deadline (due step, entry[0]) is within 2 steps — a missed
            # deadline means a consumer reads an unwritten tile region
            # (the tile framework orders by program order, so that is
            # silent garbage, not a stall).
            npop = 1
            if len(pending) > 3 * steps_left:
                npop = 3
            elif len(pending) > 1.5 * steps_left:
                npop = 2
            popped = 0

            def due_pressure():
                if cur_step is None:
                    return False
                dues = [d for d, _ in pending if d is not None]
                return bool(dues) and min(dues) <= cur_step

            while pending and (popped < npop or due_pressure()):
                if popped < npop:
                    due, fn = pending.pop(0)
                else:
                    # deadline-forced: extract the first DUE unit itself
                    # (scan order keeps paired units ordered) — popping
                    # from the head here would dump the entire backlog
                    # sitting in front of a late-queue due unit
                    idx = next(i for i, (dd, _) in enumerate(pending)
                               if dd is not None and dd <= cur_step)
                    due, fn = pending.pop(idx)
                fn()
                popped += 1

        op_tiles = {}

        def emit_outproj(ctxT_b, bb, m, ch, half, tail=False):
            # 256-col out-projection matmul units; evacuation + outT DMA are
            # coalesced at 512-col granularity on the half-1 unit.
            ms = slice(m * 128, (m + 1) * 128)
            cs = slice(ch * 512 + half * 256, ch * 512 + (half + 1) * 256)
            if half == 0:
                op = ps_wk.tile([128, 512], f32, tag="wk")
                op_tiles[(bb, m, ch)] = op
            else:
                op = op_tiles.pop((bb, m, ch))
            nc.tensor.matmul(op[:, half * 256:(half + 1) * 256],
                             wo_sb[:, ms], ctxT_b[:, cs],
                             start=True, stop=True)
            if half == 1:
                ot = opool.tile([128, 512], bf16, tag="ot")
                with nc.allow_low_precision(reason="bf16 partial out"):
                    if tail and m % 2 == 0:
                        nc.scalar.copy(ot[:], op[:])
                    else:
                        nc.vector.tensor_copy(ot[:], op[:])
                nc.sync.dma_start(
                    outT[bb, ms, ch * 512:(ch + 1) * 512], ot[:])

        def proj_sub(w_sb, xts, n, sub, evac):
            # a 256-col projection sub-chunk: full K accumulation into an
            # independent PSUM tile, then a custom evacuation
            cs = slice(n * 512 + sub * 256, n * 512 + (sub + 1) * 256)
            pt = ps_wk.tile([128, 256], f32, tag="wk")
            for k in range(KT):
                nc.tensor.matmul(pt[:], w_sb[:, k, :], xts[k][:, cs],
                                 start=(k == 0), stop=(k == KT - 1))
            evac(pt, cs)

        def evac_v(dst, eng="dve"):
            def f(pt, cs):
                if eng == "act":
                    nc.scalar.copy(dst[:, cs], pt[:])
                else:
                    nc.vector.tensor_copy(dst[:, cs], pt[:])
            return f

        def evac_k8(dst):
            # k: single fp8 plane (stationary side, plane-broadcast)
            def f(pt, cs):
                with nc.allow_low_precision(reason="k fp8"):
                    nc.vector.tensor_copy(dst[:, cs], pt[:])
            return f

        def evac_q8(dst, hi_eng="dve"):
            # q: fp8 hi + fp8 residual lo in the two DoubleRow planes.
            # hi_eng="act" shortens the startup DVE chain (ACT idle there).
            def f(pt, cs):
                with nc.allow_low_precision(reason="q fp8 hi/lo"):
                    if hi_eng == "act":
                        nc.scalar.copy(dst[:, 0, cs], pt[:])
                    else:
                        nc.vector.tensor_copy(dst[:, 0, cs], pt[:])
                    nc.vector.scalar_tensor_tensor(
                        dst[:, 1, cs], pt[:], 1.0, dst[:, 0, cs], MUL, SUB)
            return f

        def proj_chunk(w_sb, xts, n, evac):
            proj_sub(w_sb, xts, n, 0, evac)
            proj_sub(w_sb, xts, n, 1, evac)

        def chunk_units(w_sb, xts, n, evac, due=None):
            # per-256-col-sub units (~0.9us PE each) with STAGGERED
            # deadlines: units sharing a due step pop in one iteration and
            # the burst delays the next QK behind ~2us of filler
            d1 = None if due is None else due + 1
            return [(due, lambda: proj_sub(w_sb, xts, n, 0, evac)),
                    (d1, lambda: proj_sub(w_sb, xts, n, 1, evac))]

        def vproj_tile(g, xts, sk):
            # v projection, transposed at the source: x sk-tile is the
            # STATIONARY operand so the PSUM result is [token, vfeature] —
            # no separate transpose pass, and the evacuation IS the va fill
            pv = ps_wk.tile([128, 128], f32, tag="wk")
            ts_ = slice(sk * 128, (sk + 1) * 128)
            for k in range(KT):
                nc.tensor.matmul(pv[:], xts[k][:, ts_], wv_sb[:, k, :],
                                 start=(k == 0), stop=(k == KT - 1))
            va = va_tiles[(g, sk)]
            dst = va[:, 0:2 * (HD + 1)].rearrange(
                "p (h c) -> p h c", h=2)[:, :, 0:HD]
            srcv = pv[:].rearrange("p (h c) -> p h c", h=2)
            with nc.allow_low_precision(reason="va f32r evac"):
                nc.vector.tensor_copy(dst, srcv)

        def v_units(g, xts, n, due=None):
            # one unit per 128-token tile; va(sk) is read by PV(sk) at
            # iteration sk+1
            return [(None if due is None else max(due + i, 0),
                     lambda s=4 * n + i: vproj_tile(g, xts, s))
                    for i in range(4)]

        # --- batch 0 input DMAs. Each DMA trigger costs ~0.63us of the
        # shared HWDGE descriptor generator, so x loads are merged: all 8
        # k-tiles of a 512-col chunk land in ONE DMA into a [128, KT, s]
        # tile (chunk 0 first). The scalar (ACT) ring carries only wv+wq
        # (anything more delays the first exp); the rest rides the idle
        # SP ring.
        def xbig_tiles(gen):
            xb = xpool.tile([128, KT, s], bf16, tag=f"x_{gen}",
                            name=f"x_{gen}")
            cols = [xb[:, k:k + 1, :].rearrange("p k c -> p (k c)")
                    for k in range(KT)]
            return xb, cols

        xbig0, xts0 = xbig_tiles(0)
        xsrc0 = xT[0].rearrange("(k p) c -> p k c", p=128)
        # chunk 0 in k-tile PAIRS so the exposed k-projection starts on the
        # first pair while later pairs transfer; chunks 1-3 merged (one
        # HWDGE slot each, off the critical path)
        for k in range(0, KT, 2):
            nc.sync.dma_start(xbig0[:, k:k + 2, 0:512],
                              xsrc0[:, k:k + 2, 0:512])
        for c in range(1, NCH):
            cs = slice(c * 512, (c + 1) * 512)
            nc.sync.dma_start(xbig0[:, :, cs], xsrc0[:, :, cs])
        nc.sync.dma_start(wo_sb[:], woT)

        def qkv_tiles(g):
            return (qkv.tile([128, 2, s], fp8, tag=f"q_{g}", name=f"q{g}"),
                    qkv.tile([128, s], fp8, tag=f"k_{g}", name=f"k{g}"))

        prepared = {0: (xts0,) + qkv_tiles(0)}

        for b in range(B):
            g = b % 2
            xts, qT8, kT8 = prepared.pop(b)

            if b == 0:
                # batch 0: expose ONLY what the first QK pair needs (k sub0
                # covers key cols 0-255, q sub0+sub1 cover the first
                # q-block issued with one-step lookahead); everything else
                # is deadline-injected into the first q-block's sk loop.
                proj_sub(wk_sb, xts, 0, 0, evac_k8(kT8))
                proj_sub(wq_sb, xts, 0, 0, evac_q8(qT8))
                proj_sub(wq_sb, xts, 0, 1, evac_q8(qT8))
                pending.append((0, lambda: proj_sub(wk_sb, xts, 0, 1,
                                                    evac_k8(kT8))))
                pending.extend(v_units(g, xts, 0, due=0))
                for n in range(1, NCH):
                    # deadlines: QK(4n) is issued at iteration 4n-1 and
                    # PV(4n) runs at iteration 4n+1
                    pending.extend(chunk_units(wk_sb, xts, n, evac_k8(kT8),
                                               due=max(4 * n - 5, 0)))
                    pending.extend(v_units(g, xts, n, due=4 * n - 2))

            # --- prepare batch b+1: emit its x DMAs at this batch's qb 0
            # (split across both rings) and enqueue its projections as
            # injectable units for this batch's attention loop.
            if b + 1 < B:
                g1 = (b + 1) % 2
                nxbig, nxts = xbig_tiles(g1)

                def emit_next_x(bb=b + 1, nxbig=nxbig):
                    xsrc = xT[bb].rearrange("(k p) c -> p k c", p=128)
                    nc.sync.dma_start(nxbig[:, :, 0:s // 2],
                                      xsrc[:, :, 0:s // 2])
                    nc.sync.dma_start(nxbig[:, :, s // 2:s],
                                      xsrc[:, :, s // 2:s])
                next_x_emitters.append(emit_next_x)
                # batch b is PE-bound (it hosts most injected work) while
                # batch b+1's attention has idle PE per step; so only what
                # b+1's FIRST steps need is prepaid during b — chunk 0 of
                # k/v/q — and chunks 1-3 are deferred into b+1's own first
                # q-block with per-step deadlines (next_late).
                next_units = []
                nq8, nk8 = qkv_tiles(g1)
                prepared[b + 1] = (nxts, nq8, nk8)

                for n in range(2):
                    next_units.extend(chunk_units(wk_sb, nxts, n,
                                                  evac_k8(nk8)))
                for n in range(2):
                    next_units.extend(v_units(g1, nxts, n))
                next_units.extend(chunk_units(wq_sb, nxts, 0, evac_q8(nq8)))
                for n in range(2, NCH):
                    next_late[0].extend(chunk_units(
                        wk_sb, nxts, n, evac_k8(nk8), due=4 * n - 5))
                    next_late[0].extend(v_units(g1, nxts, n,
                                                due=4 * n - 2))

            # --- attention: per q-block of 512 columns. QK is fp8e4
            # DoubleRow: stationary k8 [64,2,128] (stride-0 plane bcast),
            # moving q8 [64,2,256] (hi/lo planes); one exp covers both heads.
            ctxT = cpool.tile([128, s], f32r, tag=f"ctxT_{g}")

            def issue_qk(qb_, sk):
                # QK (fp8 DoubleRow) for step sk of q-block qb_
                sps = ps_sc.tile([128, 2 * 512], f32, tag="sc")
                for h in range(HPC):
                    hr = slice(h * HD, (h + 1) * HD)
                    lhsT = kT8[hr, sk * 128:(sk + 1) * 128] \
                        .unsqueeze(1).broadcast_to([HD, 2, 128])
                    for hf in range(2):
                        ms = slice(h * 512 + hf * 256,
                                   h * 512 + (hf + 1) * 256)
                        qss = slice(qb_ * 512 + hf * 256,
                                    qb_ * 512 + (hf + 1) * 256)
                        nc.tensor.matmul(
                            sps[:, ms], lhsT, qT8[hr, :, qss],
                            start=True, stop=True, perf_mode=DR)
                return sps

            def issue_exp(sps):
                et = epool.tile([128, 2 * 512], f32r, tag="exp")
                nc.scalar.activation(et[:], sps[:], Exp, scale=EXPSCALE)
                return et
            carry_et = [None]
            for qb in range(NQB):
                if qb == 0:
                    for em in next_x_emitters:
                        em()
                    next_x_emitters.clear()
                    if b > 0 and next_late[0]:
                        pos = min(2, len(pending))
                        pending[pos:pos] = next_late[0]
                        next_late[0] = []
                if qb == 1 and b + 1 < B:
                    pending.extend(next_units)
                if b == B - 1 and qb == NQB - 1 and op_carry[0]:
                    # last q-block: the previous block's carried outproj
                    # units must drain inside this loop, not in the tail
                    pending.extend(op_carry[0])
                    op_carry[0] = []
                if qb + 1 < NQB:
                    # QK(qb+1, 0) is pre-issued at this qb's iteration
                    # SK-1. Appended at the END: the deadline machinery
                    # pulls these at iters 11-12; putting them at the front
                    # made base-rate pops run them at iters 3-5, stacking
                    # ~1.8us of PE on the loaded early-block steps.
                    pending.append((11, lambda nn=qb + 1: proj_sub(
                        wq_sb, xts, nn, 0, evac_q8(qT8))))
                    pending.append((12, lambda nn=qb + 1: proj_sub(
                        wq_sb, xts, nn, 1, evac_q8(qT8))))
                qs = slice(qb * 512, (qb + 1) * 512)
                cpss = []
                for h in range(HPC):
                    cph = ps_ctx.tile([HD + 1, 512], f32, tag=f"ctx{h}")
                    cpss.append(cph)
                ets = {}
                sps_cur = None      # QK issued one step ahead of its exp
                sps_next0 = None    # QK for (qb+1, 0), issued at this tail
                for sk in range(SK + 1):
                    if sk < SK:
                        # exp issues FIRST (ACT is the step pacer), QK for
                        # the NEXT step right after, so neither ever waits
                        # behind injected filler work. Step 0 may have been
                        # pre-issued at the previous q-block's tail.
                        if sk == 0 and carry_et[0] is not None:
                            ets[0] = carry_et[0]
                            carry_et[0] = None
                        else:
                            src = sps_cur if sps_cur is not None \
                                else issue_qk(qb, sk)
                            sps_cur = None
                            ets[sk] = issue_exp(src)
                        if sk + 1 < SK:
                            sps_cur = issue_qk(qb, sk + 1)
                        elif qb + 1 < NQB:
                            sps_next0 = issue_qk(qb + 1, 0)
                    elif sps_next0 is not None:
                        carry_et[0] = issue_exp(sps_next0)
                        sps_next0 = None
                    if sk >= 1:
                        # PV lags QK by one step
                        et1 = ets.pop(sk - 1)
                        for h in range(HPC):
                            nc.tensor.matmul(
                                cpss[h][:],
                                va_tiles[(g, sk - 1)][:, h * (HD + 1):
                                                      (h + 1) * (HD + 1)],
                                et1[:, h * 512:(h + 1) * 512],
                                start=(sk - 1 == 0), stop=(sk - 1 == SK - 1))
                    pop_units(max((NQB - 1 - qb) * SK + SK - sk, 1),
                              cur_step=sk)

                # --- normalize tail per head: evacuate ctx+denom (frees the
                # PSUM bank), partition-broadcast the RAW denominator with a
                # K=1 matmul, reciprocate the [64,512] broadcast, multiply.
                is_last_qb_ = (b == B - 1 and qb == NQB - 1)
                cus = []
                for h in range(HPC):
                    cu = spool.tile([HD + 1, 512], f32r, tag=f"cu{h}",
                                    name=f"cu{h}")
                    with nc.allow_low_precision(reason="ctx evac bit-copy"):
                        # last q-block: ACT is idle after the final exp, so
                        # the two evacuations run in parallel on ACT+DVE
                        if is_last_qb_ and h == 1:
                            nc.scalar.copy(cu[:], cpss[h][:])
                        else:
                            nc.vector.tensor_copy(cu[:], cpss[h][:])
                    cus.append(cu)

                def norm_unit(h, qs=qs, cus=cus, ctxT=ctxT,
                              tail_qb=None):
                    cu = cus[h]
                    bc = ps_wk.tile([128, 512], f32, tag="wk")
                    nc.tensor.matmul(bc[0:HD, :], ones65[HD:HD + 1, :],
                                     cu[HD:HD + 1, :], start=True, stop=True)
                    rbc = spool.tile([HD, 512], f32r, tag=f"rbc{h}")
                    with nc.allow_low_precision(reason="softmax 1/denom"):
                        nc.vector.reciprocal(rbc[:], bc[0:HD, :])
                        if h == 0:
                            nc.vector.tensor_mul(ctxT[0:HD, qs],
                                                 cu[0:HD, :], rbc[:])
                        else:
                            cn = spool.tile([HD, 512], f32r, tag="cn")
                            nc.vector.tensor_mul(cn[:], cu[0:HD, :], rbc[:])
                            sh = ps_wk.tile([128, 512], f32, tag="wk")
                            nc.tensor.matmul(sh[:], shf64[:], cn[:],
                                             start=True, stop=True)
                            if tail_qb:
                                nc.scalar.copy(ctxT[HD:2 * HD, qs],
                                               sh[HD:2 * HD, :])
                            else:
                                nc.vector.tensor_copy(ctxT[HD:2 * HD, qs],
                                                      sh[HD:2 * HD, :])

                is_last_qb = (b == B - 1 and qb == NQB - 1)
                norm_units = [(h, lambda h=h: norm_unit(
                                  h, tail_qb=is_last_qb_))
                              for h in range(HPC)]
                # outproj units carry soft deadlines so they drain MID-loop:
                # otherwise their DVE evacuations queue up ahead of the
                # final normalize chain and stretch the tail
                op_units = [
                    (6 + (2 * m + hf) // 2,
                     lambda c=ctxT, bb=b, mm=m, cc=qb, hf=hf, tl=is_last_qb:
                     emit_outproj(c, bb, mm, cc, hf, tail=tl))
                    for m in range(KT) for hf in range(2)]
                if is_last_qb:
                    for _, u in norm_units + pending + op_carry[0] + op_units:
                        u()
                    pending = []
                    op_carry[0] = []
                else:
                    pending = (norm_units + op_carry[0] + pending
                               + op_units[:8])
                    op_carry[0] = op_units[8:]


_CACHE = {}


def _get_compiled(s=S, d=D, reps=1):
    key = (s, d, reps)
    if key not in _CACHE:
        import concourse.bacc as bacc
        import concourse.tile as tile
        import concourse.mybir as mybir

        f32 = mybir.dt.float32
        f32r = mybir.dt.float32r
        bf16 = mybir.dt.bfloat16
        nc = bacc.Bacc("TRN2", target_bir_lowering=False, debug=False)
        xT = nc.dram_tensor("xT", [B, d, s], bf16, kind="ExternalInput")
        wqT = nc.dram_tensor("wqT", [128, d // 128, FPC], bf16,
                             kind="ExternalInput")
        wkT = nc.dram_tensor("wkT", [128, d // 128, FPC], bf16,
                             kind="ExternalInput")
        wvT = nc.dram_tensor("wvT", [128, d // 128, FPC], bf16,
                             kind="ExternalInput")
        woT = nc.dram_tensor("woT", [FPC, d], f32r, kind="ExternalInput")
        outT = nc.dram_tensor("outT", [B, d, s], bf16,
                              kind="ExternalOutput")
        with tile.TileContext(nc) as tc:
            for _ in range(reps):
                build_mha_kernel(tc, outT.ap(), xT.ap(), wqT.ap(), wkT.ap(),
                                 wvT.ap(), woT.ap(), s=s, d=d)
        nc.compile()
        _CACHE[key] = nc
    return _CACHE[key]


def make_in_maps(x, Wq, Wk, Wv, Wo):
    """Host-side shard prep: transpose x, slice + transpose weights per core."""
    import ml_dtypes
    b, s, d = x.shape
    xT = np.ascontiguousarray(x.transpose(0, 2, 1)).astype(ml_dtypes.bfloat16)

    def prearr(wt):
        # [d, FPC] -> [128, d//128, FPC] so each SBUF partition row is one
        # contiguous DMA line (avoids 512B-descriptor strided reads)
        return np.ascontiguousarray(
            wt.reshape(d // 128, 128, FPC).transpose(1, 0, 2)).astype(
                ml_dtypes.bfloat16)

    in_maps = []
    for c in range(NCORES):
        if (c + 1) * FPC > d:
            # small-D sim configs: fewer head-slices than cores
            in_maps.append(in_maps[0])
            continue
        rs = slice(c * FPC, (c + 1) * FPC)
        in_maps.append({
            "xT": xT,
            "wqT": prearr((Wq[rs, :] * np.float32(QKSCALE)).T
                          .astype(np.float32)),
            "wkT": prearr((Wk[rs, :] * np.float32(QKSCALE)).T
                          .astype(np.float32)),
            "wvT": prearr(Wv[rs, :].T.astype(np.float32)),
            "woT": np.ascontiguousarray(Wo[:, rs].T).astype(np.float32),
        })
    return in_maps


_RUNNER = None
_RUNNER_STATE = {}


def _get_runner():
    """Build (once) a cached jitted SPMD executor mirroring
    bass2jax.run_bass_via_pjrt's multi-core path."""
    global _RUNNER
    if _RUNNER is None:
        import jax
        import jax.numpy as jnp
        from jax.sharding import Mesh, PartitionSpec, NamedSharding
        from jax.experimental.shard_map import shard_map
        import concourse.mybir as mybir
        from concourse import bass2jax

        nc = _get_compiled()
        bass2jax.install_neuronx_cc_hook()

        partition_name = (nc.partition_id_tensor.name
                          if nc.partition_id_tensor else None)
        in_names = []
        out_names = []
        out_avals = []
        for alloc in nc.m.functions[0].allocations:
            if not isinstance(alloc, mybir.MemoryLocationSet):
                continue
            name = alloc.memorylocations[0].name
            if alloc.kind == "ExternalInput":
                if name != partition_name:
                    in_names.append(name)
            elif alloc.kind == "ExternalOutput":
                out_names.append(name)
                out_avals.append(jax.core.ShapedArray(
                    tuple(alloc.tensor_shape), mybir.dt.np(alloc.dtype)))
        n_outs = len(out_names)
        all_names = in_names + out_names
        if partition_name is not None:
            all_names = all_names + [partition_name]

        def _body(*args):
            operands = list(args)
            if partition_name is not None:
                operands.append(bass2jax.partition_id_tensor())
            outs = bass2jax._bass_exec_p.bind(
                *operands,
                out_avals=tuple(out_avals),
                in_names=tuple(all_names),
                out_names=tuple(out_names),
                lowering_input_output_aliases=(),
                sim_require_finite=True,
                sim_require_nnan=True,
                nc=nc,
            )
            return tuple(outs)

        devices = jax.devices()[:NCORES]
        mesh = Mesh(np.asarray(devices), ("core",))
        # xT is identical on every core: replicate it instead of concatenating
        # 8 copies on the host.
        in_specs = tuple(PartitionSpec() if name == "xT" else PartitionSpec("core")
                         for name in in_names)
        sharded = jax.jit(
            shard_map(_body, mesh=mesh,
                      in_specs=in_specs + (PartitionSpec("core"),) * n_outs,
                      out_specs=(PartitionSpec("core"),) * n_outs,
                      check_rep=False),
            keep_unused=True)

        # separate jit: on-device sum of the 8 per-core partials (all-reduce)
        def _reduce(a):
            return jnp.sum(a.reshape((NCORES,) + tuple(out_avals[0].shape))
                           .astype(jnp.float32), axis=0)
        reduce_jit = jax.jit(_reduce)

        out_shapes = [tuple(a.shape) for a in out_avals]
        out_dtypes = [a.dtype for a in out_avals]
        zeros_dev = [None]

        rep_shd = NamedSharding(mesh, PartitionSpec())

        def call(in_maps):
            args = []
            for name in in_names:
                if name == "xT":
                    # one host->device transfer, then device-side broadcast
                    xd = jax.device_put(np.asarray(in_maps[0][name]),
                                        devices[0])
                    args.append(jax.device_put(xd, rep_shd))
                else:
                    args.append(np.concatenate(
                        [np.asarray(m[name]) for m in in_maps], axis=0))
            if zeros_dev[0] is None:
                shd = NamedSharding(mesh, PartitionSpec("core"))
                zeros_dev[0] = [
                    jax.device_put(
                        np.zeros((NCORES * sh[0],) + sh[1:], dt), shd)
                    for sh, dt in zip(out_shapes, out_dtypes)]
            outs = sharded(*args, *zeros_dev[0])
            try:
                summed = np.asarray(reduce_jit(outs[0]))
            except Exception:
                # device reduce unavailable: fetch partials, sum on host
                a = np.asarray(outs[0])
                summed = a.reshape((NCORES,) + tuple(out_avals[0].shape)).sum(0)
            return {out_names[0]: summed}

        _RUNNER_STATE.update(sharded=sharded, in_names=in_names,
                             out_shapes=out_shapes, out_dtypes=out_dtypes,
                             call=call, mesh=mesh)
        _RUNNER = call
    return _RUNNER


def run(x, Wq, Wk, Wv, Wo, bo, trace=False):
    from concourse._compat import axon_active
    in_maps = make_in_maps(x, Wq, Wk, Wv, Wo)
    if axon_active():
        summed = _get_runner()(in_maps)
        acc = summed["outT"].astype(np.float64)
        results = summed
    else:
        # native /dev/neuron* path (non-axon environments)
        from concourse import bass_utils
        r = bass_utils.run_bass_kernel_spmd(
            _get_compiled(), in_maps, core_ids=list(range(NCORES)), trace=trace)
        results = r.results
        acc = np.zeros((B, D, S), dtype=np.float64)
        for c in range(NCORES):
            acc += results[c]["outT"]
    out = acc.transpose(0, 2, 1) + np.asarray(bo, dtype=np.float64)
    return out.astype(np.float32), results


def kernel(x, Wq, Wk, Wv, Wo, bo):
    out, _ = run(np.asarray(x), np.asarray(Wq), np.asarray(Wk),
                 np.asarray(Wv), np.asarray(Wo), np.asarray(bo))
    return out


# revision 67
# speedup vs baseline: 1.0114x; 1.0114x over previous
"""Multi-head attention (B=2, S=2048, D=1024, H=16, no mask) on 8 TRN2 cores.

Sharding: tensor-parallel over heads — 2 heads per core. Each core computes
its heads' QKV projections, attention, and a partial out-projection
(row-sharded Wo); the 8 partials are summed on device and the host adds bo.

v3 (~182us TimelineSim vs the ~232us v2 baseline, rel err ~8e-3):
  - QK runs in fp8e4 DoubleRow perf mode (0.5 PE cycles/column): k is a
    single-fp8 stationary duplicated across the two DoubleRow planes via
    a stride-0 AP; q is stored hi/lo (fp8 value + fp8 residual) in the
    two moving planes, so the product is k8^T(q_hi+q_lo) — only k
    carries fp8 noise (~1.2% through softmax). q/k are scaled 4x into
    the e4m3 sweet range; 1/(sqrt(HD)*16) folds into the ACT exp scale.
  - the sk loop is software-pipelined: exp(sk) issues first, QK(sk+1)
    right after (one step of lookahead), then PV(sk-1), then injected
    filler; the next q-block's first QK+exp are pre-issued at this
    block's tail so ACT (the pacer: 128 x ~1.04us exps) never starves.
  - filler work (projections, va builds, outproj, normalize) is a
    deadline-tagged unit queue: pop_units force-pops anything due at the
    current step (a missed deadline = a consumer reading an unwritten
    tile region = silent garbage), and paces the rest across the batch.
  - v is projected TRANSPOSED (x sk-tile as the stationary operand), so
    the [token, vfeature] PSUM result fills the ones-augmented va tiles
    with a single strided copy — no transpose pass.
  - x loads: k-tile-pair DMAs for chunk 0 (pipelines into the exposed
    projections), merged whole-row DMAs for chunks 1-3 and batch 1; the
    scalar (ACT) ring carries only wv+wq (each DMA trigger costs ~0.6us
    of the shared HWDGE generator and would delay the first exp).
  - softmax normalize: ctx PSUM is evacuated once per head (frees the
    bank), the raw denominator row is partition-broadcast with one K=1
    PE matmul, reciprocated as a [64,512] DVE op (accurate on positive
    denominators), and multiplied in. Out-projection partials ship bf16.
"""
import numpy as np

B = 2
S = 2048
D = 1024
H = 16
HD = 64
NCORES = 8
HPC = H // NCORES       # heads per core
FPC = HPC * HD          # 128 features per core
QKSCALE = 4.0           # q/k fp8 range scaling (folded out in exp scale)
EXPSCALE = 1.0 / (8.0 * QKSCALE * QKSCALE)


def build_mha_kernel(tc, outT, xT, wqT, wkT, wvT, woT, s=S, d=D):
    """Emit the per-core MHA program.

    outT: [B, d, s] f32 (partial output, transposed, per-batch)
    xT:   [B, d, s] bf16 (host pre-transposed)
    wqT/wkT/wvT: [128, d//128, FPC] bf16, host pre-arranged so the
        weight DMA is contiguous (wqT/wkT pre-scaled by QKSCALE)
    woT:  [FPC, d] f32r
    """
    import concourse.mybir as mybir
    from concourse.masks import make_identity
    from contextlib import ExitStack

    nc = tc.nc
    f32 = mybir.dt.float32
    f32r = mybir.dt.float32r
    bf16 = mybir.dt.bfloat16
    fp8 = mybir.dt.float8e4
    Exp = mybir.ActivationFunctionType.Exp
    DR = mybir.MatmulPerfMode.DoubleRow
    MUL = mybir.AluOpType.mult
    SUB = mybir.AluOpType.subtract

    KT = d // 128           # contraction tiles for projections
    SK = s // 128           # key tiles
    NQB = s // 512          # 512-col query blocks
    NCH = s // 512          # 512-token chunks

    with ExitStack() as es:
        consts = es.enter_context(tc.tile_pool(name="consts", bufs=1))
        wpool = es.enter_context(tc.tile_pool(name="w", bufs=1))
        xpool = es.enter_context(tc.tile_pool(name="xt", bufs=1))
        qkv = es.enter_context(tc.tile_pool(name="qkv", bufs=1))
        vapool = es.enter_context(tc.tile_pool(name="va", bufs=1))
        epool = es.enter_context(tc.tile_pool(name="exp", bufs=3))
        cpool = es.enter_context(tc.tile_pool(name="ctxT", bufs=1))
        spool = es.enter_context(tc.tile_pool(name="small", bufs=2))
        opool = es.enter_context(tc.tile_pool(name="o", bufs=6))
        ps_ctx = es.enter_context(tc.tile_pool(name="psctx", bufs=1, space="PSUM"))
        ps_sc = es.enter_context(tc.tile_pool(name="pssc", bufs=2, space="PSUM"))
        ps_wk = es.enter_context(tc.tile_pool(name="pswk", bufs=2, space="PSUM"))

        identity = consts.tile([128, 128], f32, tag="ident")
        make_identity(nc, identity[:])
        ident_r = consts.tile([128, 128], f32r, tag="ident_r")
        nc.vector.tensor_copy(ident_r[:], identity[:])
        ones_f = consts.tile([128, 1], f32, tag="ones_f")
        nc.gpsimd.memset(ones_f[:], 1.0)
        # bc stationary: row 64 of a [65, 64] tile is all-ones so the K=1
        # denominator-broadcast matmul's lhsT sits at partition 64 (same
        # base partition as the denominator row in cu).
        ones65_f = consts.tile([65, HD], f32, tag="ones65_f")
        nc.gpsimd.memset(ones65_f[:], 1.0)
        ones65 = consts.tile([65, HD], f32r, tag="ones65")
        nc.vector.tensor_copy(ones65[:], ones65_f[:])
        # shf64: sh[64+j,q] = cn[j,q]  (normalized h1 ctx -> lanes 64-127)
        shf_f = consts.tile([HD, 2 * HD], f32, tag="shf_f")
        nc.gpsimd.memset(shf_f[:], 0.0)
        nc.vector.tensor_copy(shf_f[0:HD, HD:2 * HD], identity[0:HD, 0:HD])
        shf64 = consts.tile([HD, 2 * HD], f32r, tag="shf64")
        nc.vector.tensor_copy(shf64[:], shf_f[:])

        # --- weights (resident for the whole kernel). wk/wv go first on the
        # two HWDGE rings; wq after batch 0's chunk-0 x tiles; wo later.
        wk_sb = wpool.tile([128, KT, FPC], bf16, tag="wk")
        nc.sync.dma_start(wk_sb[:], wkT)
        wv_sb = wpool.tile([128, KT, FPC], bf16, tag="wv")
        nc.scalar.dma_start(wv_sb[:], wvT)
        wq_sb = wpool.tile([128, KT, FPC], bf16, tag="wq")
        # wq trigger rides early on the scalar ring: HWDGE serves rings
        # round-robin, so a late wq sits behind the x tails and stalls the
        # exposed q projection (~2us of PE idle at startup)
        nc.scalar.dma_start(wq_sb[:], wqT)
        wo_sb = wpool.tile([128, d], f32r, tag="wo")

        # --- PE p-state warm-up: the PE runs at mid speed until it has a
        # ~3us continuous execution streak. These dummy matmuls burn the
        # otherwise-idle DMA-wait window so the exposed chunk-0 projections
        # (and everything after) run at full speed sooner.
        warm = ps_wk.tile([128, 128], f32, tag="wk")
        for _ in range(14):
            nc.tensor.matmul(warm[:], ident_r[:], ident_r[:],
                             start=True, stop=True)

        # --- persistent ones-augmented v tiles, both heads per tile:
        # columns [h0 v(64) | ones | h1 v(64) | ones].
        va_tiles = {}
        for g in range(2):
            for sk in range(SK):
                va = vapool.tile([128, 2 * (HD + 1)], f32r,
                                 tag=f"va{sk}_{g}", name=f"va{sk}_{g}")
                for h in range(HPC):
                    # on Pool: 64 tiny copies would clog the DVE queue head
                    # and delay the chunk-0 projection evacuations
                    nc.gpsimd.tensor_copy(
                        va[:, h * (HD + 1) + HD:h * (HD + 1) + HD + 1],
                        ones_f[:])
                va_tiles[(g, sk)] = va

        pending = []
        op_carry = [[]]
        next_late = [[]]
        next_x_emitters = []

        def pop_units(steps_left, cur_step=None):
            # pop at an adaptive base rate, but ALWAYS pop units whose
            # deadline (due step, entry[0]) is within 2 steps — a missed
            # deadline means a consumer reads an unwritten tile region
            # (the tile framework orders by program order, so that is
            # silent garbage, not a stall).
            npop = 1
            if len(pending) > 3 * steps_left:
                npop = 3
            elif len(pending) > 1.5 * steps_left:
                npop = 2
            popped = 0

            def due_pressure():
                if cur_step is None:
                    return False
                dues = [d for d, _ in pending if d is not None]
                return bool(dues) and min(dues) <= cur_step

            while pending and (popped < npop or due_pressure()):
                if popped < npop:
                    due, fn = pending.pop(0)
                else:
                    # deadline-forced: extract the first DUE unit itself
                    # (scan order keeps paired units ordered) — popping
                    # from the head here would dump the entire backlog
                    # sitting in front of a late-queue due unit
                    idx = next(i for i, (dd, _) in enumerate(pending)
                               if dd is not None and dd <= cur_step)
                    due, fn = pending.pop(idx)
                fn()
                popped += 1

        op_tiles = {}

        def emit_outproj(ctxT_b, bb, m, ch, half, tail=False):
            # 256-col out-projection matmul units; evacuation + outT DMA are
            # coalesced at 512-col granularity on the half-1 unit.
            ms = slice(m * 128, (m + 1) * 128)
            cs = slice(ch * 512 + half * 256, ch * 512 + (half + 1) * 256)
            if half == 0:
                op = ps_wk.tile([128, 512], f32, tag="wk")
                op_tiles[(bb, m, ch)] = op
            else:
                op = op_tiles.pop((bb, m, ch))
            nc.tensor.matmul(op[:, half * 256:(half + 1) * 256],
                             wo_sb[:, ms], ctxT_b[:, cs],
                             start=True, stop=True)
            if half == 1:
                ot = opool.tile([128, 512], bf16, tag="ot")
                with nc.allow_low_precision(reason="bf16 partial out"):
                    if tail and m % 2 == 0:
                        nc.scalar.copy(ot[:], op[:])
                    else:
                        nc.vector.tensor_copy(ot[:], op[:])
                nc.sync.dma_start(
                    outT[bb, ms, ch * 512:(ch + 1) * 512], ot[:])

        def proj_sub(w_sb, xts, n, sub, evac):
            # a 256-col projection sub-chunk: full K accumulation into an
            # independent PSUM tile, then a custom evacuation
            cs = slice(n * 512 + sub * 256, n * 512 + (sub + 1) * 256)
            pt = ps_wk.tile([128, 256], f32, tag="wk")
            for k in range(KT):
                nc.tensor.matmul(pt[:], w_sb[:, k, :], xts[k][:, cs],
                                 start=(k == 0), stop=(k == KT - 1))
            evac(pt, cs)

        def evac_v(dst, eng="dve"):
            def f(pt, cs):
                if eng == "act":
                    nc.scalar.copy(dst[:, cs], pt[:])
                else:
                    nc.vector.tensor_copy(dst[:, cs], pt[:])
            return f

        def evac_k8(dst):
            # k: single fp8 plane (stationary side, plane-broadcast)
            def f(pt, cs):
                with nc.allow_low_precision(reason="k fp8"):
                    nc.vector.tensor_copy(dst[:, cs], pt[:])
            return f

        def evac_q8(dst, hi_eng="dve"):
            # q: fp8 hi + fp8 residual lo in the two DoubleRow planes.
            # hi_eng="act" shortens the startup DVE chain (ACT idle there).
            def f(pt, cs):
                with nc.allow_low_precision(reason="q fp8 hi/lo"):
                    if hi_eng == "act":
                        nc.scalar.copy(dst[:, 0, cs], pt[:])
                    else:
                        nc.vector.tensor_copy(dst[:, 0, cs], pt[:])
                    nc.vector.scalar_tensor_tensor(
                        dst[:, 1, cs], pt[:], 1.0, dst[:, 0, cs], MUL, SUB)
            return f

        def proj_chunk(w_sb, xts, n, evac):
            proj_sub(w_sb, xts, n, 0, evac)
            proj_sub(w_sb, xts, n, 1, evac)

        def chunk_units(w_sb, xts, n, evac, due=None):
            # per-256-col-sub units (~0.9us PE each) with STAGGERED
            # deadlines: units sharing a due step pop in one iteration and
            # the burst delays the next QK behind ~2us of filler
            d1 = None if due is None else due + 1
            return [(due, lambda: proj_sub(w_sb, xts, n, 0, evac)),
                    (d1, lambda: proj_sub(w_sb, xts, n, 1, evac))]

        def vproj_tile(g, xts, sk):
            # v projection, transposed at the source: x sk-tile is the
            # STATIONARY operand so the PSUM result is [token, vfeature] —
            # no separate transpose pass, and the evacuation IS the va fill
            pv = ps_wk.tile([128, 128], f32, tag="wk")
            ts_ = slice(sk * 128, (sk + 1) * 128)
            for k in range(KT):
                nc.tensor.matmul(pv[:], xts[k][:, ts_], wv_sb[:, k, :],
                                 start=(k == 0), stop=(k == KT - 1))
            va = va_tiles[(g, sk)]
            dst = va[:, 0:2 * (HD + 1)].rearrange(
                "p (h c) -> p h c", h=2)[:, :, 0:HD]
            srcv = pv[:].rearrange("p (h c) -> p h c", h=2)
            with nc.allow_low_precision(reason="va f32r evac"):
                nc.vector.tensor_copy(dst, srcv)

        def v_units(g, xts, n, due=None):
            # one unit per 128-token tile; va(sk) is read by PV(sk) at
            # iteration sk+1
            return [(None if due is None else max(due + i, 0),
                     lambda s=4 * n + i: vproj_tile(g, xts, s))
                    for i in range(4)]

        # --- batch 0 input DMAs. Each DMA trigger costs ~0.63us of the
        # shared HWDGE descriptor generator, so x loads are merged: all 8
        # k-tiles of a 512-col chunk land in ONE DMA into a [128, KT, s]
        # tile (chunk 0 first). The scalar (ACT) ring carries only wv+wq
        # (anything more delays the first exp); the rest rides the idle
        # SP ring.
        def xbig_tiles(gen):
            xb = xpool.tile([128, KT, s], bf16, tag=f"x_{gen}",
                            name=f"x_{gen}")
            cols = [xb[:, k:k + 1, :].rearrange("p k c -> p (k c)")
                    for k in range(KT)]
            return xb, cols

        xbig0, xts0 = xbig_tiles(0)
        xsrc0 = xT[0].rearrange("(k p) c -> p k c", p=128)
        # chunk 0 in k-tile PAIRS so the exposed k-projection starts on the
        # first pair while later pairs transfer; chunks 1-3 merged (one
        # HWDGE slot each, off the critical path)
        for k in range(0, KT, 2):
            nc.sync.dma_start(xbig0[:, k:k + 2, 0:512],
                              xsrc0[:, k:k + 2, 0:512])
        for c in range(1, NCH):
            cs = slice(c * 512, (c + 1) * 512)
            nc.sync.dma_start(xbig0[:, :, cs], xsrc0[:, :, cs])
        nc.sync.dma_start(wo_sb[:], woT)

        def qkv_tiles(g):
            return (qkv.tile([128, 2, s], fp8, tag=f"q_{g}", name=f"q{g}"),
                    qkv.tile([128, s], fp8, tag=f"k_{g}", name=f"k{g}"))

        prepared = {0: (xts0,) + qkv_tiles(0)}

        for b in range(B):
            g = b % 2
            xts, qT8, kT8 = prepared.pop(b)

            if b == 0:
                # batch 0: expose ONLY what the first QK pair needs (k sub0
                # covers key cols 0-255, q sub0+sub1 cover the first
                # q-block issued with one-step lookahead); everything else
                # is deadline-injected into the first q-block's sk loop.
                proj_sub(wk_sb, xts, 0, 0, evac_k8(kT8))
                proj_sub(wq_sb, xts, 0, 0, evac_q8(qT8))
                proj_sub(wq_sb, xts, 0, 1, evac_q8(qT8))
                pending.append((0, lambda: proj_sub(wk_sb, xts, 0, 1,
                                                    evac_k8(kT8))))
                pending.extend(v_units(g, xts, 0, due=0))
                for n in range(1, NCH):
                    # deadlines: QK(4n) is issued at iteration 4n-1 and
                    # PV(4n) runs at iteration 4n+1
                    pending.extend(chunk_units(wk_sb, xts, n, evac_k8(kT8),
                                               due=max(4 * n - 5, 0)))
                    pending.extend(v_units(g, xts, n, due=4 * n - 2))

            # --- prepare batch b+1: emit its x DMAs at this batch's qb 0
            # (split across both rings) and enqueue its projections as
            # injectable units for this batch's attention loop.
            if b + 1 < B:
                g1 = (b + 1) % 2
                nxbig, nxts = xbig_tiles(g1)

                def emit_next_x(bb=b + 1, nxbig=nxbig):
                    xsrc = xT[bb].rearrange("(k p) c -> p k c", p=128)
                    nc.sync.dma_start(nxbig[:, :, 0:s // 2],
                                      xsrc[:, :, 0:s // 2])
                    nc.sync.dma_start(nxbig[:, :, s // 2:s],
                                      xsrc[:, :, s // 2:s])
                next_x_emitters.append(emit_next_x)
                # batch b is PE-bound (it hosts most injected work) while
                # batch b+1's attention has idle PE per step; so only what
                # b+1's FIRST steps need is prepaid during b — chunk 0 of
                # k/v/q — and chunks 1-3 are deferred into b+1's own first
                # q-block with per-step deadlines (next_late).
                next_units = []
                nq8, nk8 = qkv_tiles(g1)
                prepared[b + 1] = (nxts, nq8, nk8)

                for n in range(2):
                    next_units.extend(chunk_units(wk_sb, nxts, n,
                                                  evac_k8(nk8)))
                for n in range(2):
                    next_units.extend(v_units(g1, nxts, n))
                next_units.extend(chunk_units(wq_sb, nxts, 0, evac_q8(nq8)))
                for n in range(2, NCH):
                    next_late[0].extend(chunk_units(
                        wk_sb, nxts, n, evac_k8(nk8), due=4 * n - 5))
                    next_late[0].extend(v_units(g1, nxts, n,
                                                due=4 * n - 2))

            # --- attention: per q-block of 512 columns. QK is fp8e4
            # DoubleRow: stationary k8 [64,2,128] (stride-0 plane bcast),
            # moving q8 [64,2,256] (hi/lo planes); one exp covers both heads.
            ctxT = cpool.tile([128, s], f32r, tag=f"ctxT_{g}")

            def issue_qk(qb_, sk):
                # QK (fp8 DoubleRow) for step sk of q-block qb_
                sps = ps_sc.tile([128, 2 * 512], f32, tag="sc")
                for h in range(HPC):
                    hr = slice(h * HD, (h + 1) * HD)
                    lhsT = kT8[hr, sk * 128:(sk + 1) * 128] \
                        .unsqueeze(1).broadcast_to([HD, 2, 128])
                    for hf in range(2):
                        ms = slice(h * 512 + hf * 256,
                                   h * 512 + (hf + 1) * 256)
                        qss = slice(qb_ * 512 + hf * 256,
                                    qb_ * 512 + (hf + 1) * 256)
                        nc.tensor.matmul(
                            sps[:, ms], lhsT, qT8[hr, :, qss],
                            start=True, stop=True, perf_mode=DR)
                return sps

            def issue_exp(sps):
                et = epool.tile([128, 2 * 512], f32r, tag="exp")
                nc.scalar.activation(et[:], sps[:], Exp, scale=EXPSCALE)
                return et
            carry_et = [None]
            for qb in range(NQB):
                if qb == 0:
                    for em in next_x_emitters:
                        em()
                    next_x_emitters.clear()
                    if b > 0 and next_late[0]:
                        pos = min(2, len(pending))
                        pending[pos:pos] = next_late[0]
                        next_late[0] = []
                if qb == 1 and b + 1 < B:
                    pending.extend(next_units)
                if b == B - 1 and qb == NQB - 1 and op_carry[0]:
                    # last q-block: the previous block's carried outproj
                    # units must drain inside this loop, not in the tail
                    pending.extend(op_carry[0])
                    op_carry[0] = []
                if qb + 1 < NQB:
                    # QK(qb+1, 0) is pre-issued at this qb's iteration
                    # SK-1. Appended at the END: the deadline machinery
                    # pulls these at iters 11-12; putting them at the front
                    # made base-rate pops run them at iters 3-5, stacking
                    # ~1.8us of PE on the loaded early-block steps.
                    pending.append((11, lambda nn=qb + 1: proj_sub(
                        wq_sb, xts, nn, 0, evac_q8(qT8))))
                    pending.append((12, lambda nn=qb + 1: proj_sub(
                        wq_sb, xts, nn, 1, evac_q8(qT8))))
                qs = slice(qb * 512, (qb + 1) * 512)
                cpss = []
                for h in range(HPC):
                    cph = ps_ctx.tile([HD + 1, 512], f32, tag=f"ctx{h}")
                    cpss.append(cph)
                ets = {}
                sps_cur = None      # QK issued one step ahead of its exp
                sps_next0 = None    # QK for (qb+1, 0), issued at this tail
                for sk in range(SK + 1):
                    if sk < SK:
                        # exp issues FIRST (ACT is the step pacer), QK for
                        # the NEXT step right after, so neither ever waits
                        # behind injected filler work. Step 0 may have been
                        # pre-issued at the previous q-block's tail.
                        if sk == 0 and carry_et[0] is not None:
                            ets[0] = carry_et[0]
                            carry_et[0] = None
                        else:
                            src = sps_cur if sps_cur is not None \
                                else issue_qk(qb, sk)
                            sps_cur = None
                            ets[sk] = issue_exp(src)
                        if sk + 1 < SK:
                            sps_cur = issue_qk(qb, sk + 1)
                        elif qb + 1 < NQB:
                            sps_next0 = issue_qk(qb + 1, 0)
                    elif sps_next0 is not None:
                        carry_et[0] = issue_exp(sps_next0)
                        sps_next0 = None
                    if sk >= 1:
                        # PV lags QK by one step
                        et1 = ets.pop(sk - 1)
                        for h in range(HPC):
                            nc.tensor.matmul(
                                cpss[h][:],
                                va_tiles[(g, sk - 1)][:, h * (HD + 1):
                                                      (h + 1) * (HD + 1)],
                                et1[:, h * 512:(h + 1) * 512],
                                start=(sk - 1 == 0), stop=(sk - 1 == SK - 1))
                    pop_units(max((NQB - 1 - qb) * SK + SK - sk, 1),
                              cur_step=sk)

                # --- normalize tail per head: evacuate ctx+denom (frees the
                # PSUM bank), partition-broadcast the RAW denominator with a
                # K=1 matmul, reciprocate the [64,512] broadcast, multiply.
                is_last_qb_ = (b == B - 1 and qb == NQB - 1)
                cus = []
                for h in range(HPC):
                    cu = spool.tile([HD + 1, 512], f32r, tag=f"cu{h}",
                                    name=f"cu{h}")
                    with nc.allow_low_precision(reason="ctx evac bit-copy"):
                        # last q-block: ACT is idle after the final exp, so
                        # the two evacuations run in parallel on ACT+DVE
                        if is_last_qb_ and h == 1:
                            nc.scalar.copy(cu[:], cpss[h][:])
                        else:
                            nc.vector.tensor_copy(cu[:], cpss[h][:])
                    cus.append(cu)

                def norm_unit(h, qs=qs, cus=cus, ctxT=ctxT,
                              tail_qb=None):
                    cu = cus[h]
                    bc = ps_wk.tile([128, 512], f32, tag="wk")
                    nc.tensor.matmul(bc[0:HD, :], ones65[HD:HD + 1, :],
                                     cu[HD:HD + 1, :], start=True, stop=True)
                    rbc = spool.tile([HD, 512], f32r, tag=f"rbc{h}")
                    with nc.allow_low_precision(reason="softmax 1/denom"):
                        nc.vector.reciprocal(rbc[:], bc[0:HD, :])
                        if h == 0:
                            nc.vector.tensor_mul(ctxT[0:HD, qs],
                                                 cu[0:HD, :], rbc[:])
                        else:
                            cn = spool.tile([HD, 512], f32r, tag="cn")
                            nc.vector.tensor_mul(cn[:], cu[0:HD, :], rbc[:])
                            sh = ps_wk.tile([128, 512], f32, tag="wk")
                            nc.tensor.matmul(sh[:], shf64[:], cn[:],
                                             start=True, stop=True)
                            if tail_qb:
                                nc.scalar.copy(ctxT[HD:2 * HD, qs],
                                               sh[HD:2 * HD, :])
                            else:
                                nc.vector.tensor_copy(ctxT[HD:2 * HD, qs],
                                                      sh[HD:2 * HD, :])

                is_last_qb = (b == B - 1 and qb == NQB - 1)
                norm_units = [(h, lambda h=h: norm_unit(
                                  h, tail_qb=is_last_qb_))
                              for h in range(HPC)]
                # outproj units carry soft deadlines so they drain MID-loop:
                # otherwise their DVE evacuations queue up ahead of the
                # final normalize chain and stretch the tail
                op_units = [
                    (8 + (2 * m + hf) // 2,
                     lambda c=ctxT, bb=b, mm=m, cc=qb, hf=hf, tl=is_last_qb:
                     emit_outproj(c, bb, mm, cc, hf, tail=tl))
                    for m in range(KT) for hf in range(2)]
                if is_last_qb:
                    for _, u in norm_units + pending + op_carry[0] + op_units:
                        u()
                    pending = []
                    op_carry[0] = []
                else:
                    pending = (norm_units + op_carry[0] + pending
                               + op_units[:8])
                    op_carry[0] = op_units[8:]


_CACHE = {}


def _get_compiled(s=S, d=D, reps=1):
    key = (s, d, reps)
    if key not in _CACHE:
        import concourse.bacc as bacc
        import concourse.tile as tile
        import concourse.mybir as mybir

        f32 = mybir.dt.float32
        f32r = mybir.dt.float32r
        bf16 = mybir.dt.bfloat16
        nc = bacc.Bacc("TRN2", target_bir_lowering=False, debug=False)
        xT = nc.dram_tensor("xT", [B, d, s], bf16, kind="ExternalInput")
        wqT = nc.dram_tensor("wqT", [128, d // 128, FPC], bf16,
                             kind="ExternalInput")
        wkT = nc.dram_tensor("wkT", [128, d // 128, FPC], bf16,
                             kind="ExternalInput")
        wvT = nc.dram_tensor("wvT", [128, d // 128, FPC], bf16,
                             kind="ExternalInput")
        woT = nc.dram_tensor("woT", [FPC, d], f32r, kind="ExternalInput")
        outT = nc.dram_tensor("outT", [B, d, s], bf16,
                              kind="ExternalOutput")
        with tile.TileContext(nc) as tc:
            for _ in range(reps):
                build_mha_kernel(tc, outT.ap(), xT.ap(), wqT.ap(), wkT.ap(),
                                 wvT.ap(), woT.ap(), s=s, d=d)
        nc.compile()
        _CACHE[key] = nc
    return _CACHE[key]


def make_in_maps(x, Wq, Wk, Wv, Wo):
    """Host-side shard prep: transpose x, slice + transpose weights per core."""
    import ml_dtypes
    b, s, d = x.shape
    xT = np.ascontiguousarray(x.transpose(0, 2, 1)).astype(ml_dtypes.bfloat16)

    def prearr(wt):
        # [d, FPC] -> [128, d//128, FPC] so each SBUF partition row is one
        # contiguous DMA line (avoids 512B-descriptor strided reads)
        return np.ascontiguousarray(
            wt.reshape(d // 128, 128, FPC).transpose(1, 0, 2)).astype(
                ml_dtypes.bfloat16)

    in_maps = []
    for c in range(NCORES):
        if (c + 1) * FPC > d:
            # small-D sim configs: fewer head-slices than cores
            in_maps.append(in_maps[0])
            continue
        rs = slice(c * FPC, (c + 1) * FPC)
        in_maps.append({
            "xT": xT,
            "wqT": prearr((Wq[rs, :] * np.float32(QKSCALE)).T
                          .astype(np.float32)),
            "wkT": prearr((Wk[rs, :] * np.float32(QKSCALE)).T
                          .astype(np.float32)),
            "wvT": prearr(Wv[rs, :].T.astype(np.float32)),
            "woT": np.ascontiguousarray(Wo[:, rs].T).astype(np.float32),
        })
    return in_maps


_RUNNER = None
_RUNNER_STATE = {}


def _get_runner():
    """Build (once) a cached jitted SPMD executor mirroring
    bass2jax.run_bass_via_pjrt's multi-core path."""
    global _RUNNER
    if _RUNNER is None:
        import jax
        import jax.numpy as jnp
        from jax.sharding import Mesh, PartitionSpec, NamedSharding
        from jax.experimental.shard_map import shard_map
        import concourse.mybir as mybir
        from concourse import bass2jax

        nc = _get_compiled()
        bass2jax.install_neuronx_cc_hook()

        partition_name = (nc.partition_id_tensor.name
                          if nc.partition_id_tensor else None)
        in_names = []
        out_names = []
        out_avals = []
        for alloc in nc.m.functions[0].allocations:
            if not isinstance(alloc, mybir.MemoryLocationSet):
                continue
            name = alloc.memorylocations[0].name
            if alloc.kind == "ExternalInput":
                if name != partition_name:
                    in_names.append(name)
            elif alloc.kind == "ExternalOutput":
                out_names.append(name)
                out_avals.append(jax.core.ShapedArray(
                    tuple(alloc.tensor_shape), mybir.dt.np(alloc.dtype)))
        n_outs = len(out_names)
        all_names = in_names + out_names
        if partition_name is not None:
            all_names = all_names + [partition_name]

        def _body(*args):
            operands = list(args)
            if partition_name is not None:
                operands.append(bass2jax.partition_id_tensor())
            outs = bass2jax._bass_exec_p.bind(
                *operands,
                out_avals=tuple(out_avals),
                in_names=tuple(all_names),
                out_names=tuple(out_names),
                lowering_input_output_aliases=(),
                sim_require_finite=True,
                sim_require_nnan=True,
                nc=nc,
            )
            return tuple(outs)

        devices = jax.devices()[:NCORES]
        mesh = Mesh(np.asarray(devices), ("core",))
        # xT is identical on every core: replicate it instead of concatenating
        # 8 copies on the host.
        in_specs = tuple(PartitionSpec() if name == "xT" else PartitionSpec("core")
                         for name in in_names)
        sharded = jax.jit(
            shard_map(_body, mesh=mesh,
                      in_specs=in_specs + (PartitionSpec("core"),) * n_outs,
                      out_specs=(PartitionSpec("core"),) * n_outs,
                      check_rep=False),
            keep_unused=True)

        # separate jit: on-device sum of the 8 per-core partials (all-reduce)
        def _reduce(a):
            return jnp.sum(a.reshape((NCORES,) + tuple(out_avals[0].shape))
                           .astype(jnp.float32), axis=0)
        reduce_jit = jax.jit(_reduce)

        out_shapes = [tuple(a.shape) for a in out_avals]
        out_dtypes = [a.dtype for a in out_avals]
        zeros_dev = [None]

        rep_shd = NamedSharding(mesh, PartitionSpec())

        def call(in_maps):
            args = []
            for name in in_names:
                if name == "xT":
                    # one host->device transfer, then device-side broadcast
                    xd = jax.device_put(np.asarray(in_maps[0][name]),
                                        devices[0])
                    args.append(jax.device_put(xd, rep_shd))
                else:
                    args.append(np.concatenate(
                        [np.asarray(m[name]) for m in in_maps], axis=0))
            if zeros_dev[0] is None:
                shd = NamedSharding(mesh, PartitionSpec("core"))
                zeros_dev[0] = [
                    jax.device_put(
                        np.zeros((NCORES * sh[0],) + sh[1:], dt), shd)
                    for sh, dt in zip(out_shapes, out_dtypes)]
            outs = sharded(*args, *zeros_dev[0])
            try:
                summed = np.asarray(reduce_jit(outs[0]))
            except Exception:
                # device reduce unavailable: fetch partials, sum on host
                a = np.asarray(outs[0])
                summed = a.reshape((NCORES,) + tuple(out_avals[0].shape)).sum(0)
            return {out_names[0]: summed}

        _RUNNER_STATE.update(sharded=sharded, in_names=in_names,
                             out_shapes=out_shapes, out_dtypes=out_dtypes,
                             call=call, mesh=mesh)
        _RUNNER = call
    return _RUNNER


def run(x, Wq, Wk, Wv, Wo, bo, trace=False):
    from concourse._compat import axon_active
    in_maps = make_in_maps(x, Wq, Wk, Wv, Wo)
    if axon_active():
        summed = _get_runner()(in_maps)
        acc = summed["outT"].astype(np.float64)
        results = summed
    else:
        # native /dev/neuron* path (non-axon environments)
        from concourse import bass_utils
        r = bass_utils.run_bass_kernel_spmd(
            _get_compiled(), in_maps, core_ids=list(range(NCORES)), trace=trace)
        results = r.results
        acc = np.zeros((B, D, S), dtype=np.float64)
        for c in range(NCORES):
            acc += results[c]["outT"]
    out = acc.transpose(0, 2, 1) + np.asarray(bo, dtype=np.float64)
    return out.astype(np.float32), results


def kernel(x, Wq, Wk, Wv, Wo, bo):
    out, _ = run(np.asarray(x), np.asarray(Wq), np.asarray(Wk),
                 np.asarray(Wv), np.asarray(Wo), np.asarray(bo))
    return out
